# revision 3
# baseline (speedup 1.0000x reference)
"""Causal self-attention (B=2, T=2048, D=1024, NH=16) on 8 Trainium2 NeuronCores.

Sharding: core c handles batch b = c // 4 and heads [4*(c%4), 4*(c%4)+4)
(data parallel over B, head/tensor parallel over NH). Wqkv is column-sliced,
Wproj row-sliced per core; each core emits its partial projection
out_c = Y_c @ Wproj_c (transposed, [D, T]); the host sums the four partials
per batch and adds bproj. No cross-core collectives are needed.

v2: the whole kernel is a 4-stage software pipeline over 512-token column
spans ("quarters"). Per span sp: x pieces stream in (DMA) -> QKV projection
for that span -> attention for query quarter sp (all 4 heads) -> output
projection for span sp -> output DMA. Stages overlap: while quarter sp's
softmax exps run on ACT, the tensor engine computes QK/V for span sp+1 and
the projection for span sp. This keeps PE (the overall bottleneck) and ACT
(the attention bottleneck) both fed, and hides the input-DMA ramp behind
k-ordered QK accumulation.

On-device layout (as v1): contraction dim on SBUF partitions everywhere.
Host passes x^T so QKV^T = W^T @ x^T lands in [dim, token] layout. Attention
computes S^T (keys on partitions); exp(S^T) feeds the PV matmul straight
from SBUF; the softmax denominator comes free via ones-columns in the V
blocks (V1 layout per 128-token chunk: even head [V|1@64|pad], odd head
[pad|1@32|pad|V@64]). Causality: skip all-masked key blocks, mask the
diagonal 128x128 block with a 0/1 triangular SBUF mask. All S-tile psum
slots are packed tightly (spans start mid-bank where needed) so exp never
processes garbage columns; all but the unavoidable 128-wide diagonal
matmuls keep free dim >= 256 (fp32r runs 1/4 rate below 256).

Matmuls run in float32r (TF32-like fast fp32 mode; ~1e-3 max rel err).
PSUM budget (8 banks): 2 banks QK pair accumulators (own pool so span
sp+1's QKV can run during span sp's attention), 4 banks S tiles
(2 x [128,1024], double-buffered so S matmuls overlap the previous
group's exp), 2 banks shared V accumulators / attention outputs /
projection accumulators.
"""

import contextlib

import numpy as np

import concourse.bass as bass
import concourse.mybir as mybir
import concourse.tile as tile
from concourse import bacc
from concourse.bass_utils import run_bass_kernel_spmd

B, T, D = 2, 2048, 1024
NH, HD = 16, 64
NCORES = 8
HPC = 4                 # heads per core
HDIM = HPC * HD         # 256 qkv dims per core
KCH = D // 128          # 8 contraction chunks
NSP = 4                 # 512-token column spans
SPW = T // NSP          # 512
F32 = mybir.dt.float32
F32R = mybir.dt.float32r
EXP = mybir.ActivationFunctionType.Exp
MULT = mybir.AluOpType.mult

_NC = None


def _groups(q):
    """S-block groups for query quarter q. Each group is a list of
    (ki, s, w, off): key block ki, query span [s, s+w), psum col offset off.
    Offsets are packed so exp covers no garbage columns; every span stays
    inside a 512-col psum bank."""
    gs = []
    full = list(range(0, 4 * q))
    for i in range(0, len(full), 2):
        gs.append([(ki, SPW * q, 512, 512 * idx)
                   for idx, ki in enumerate(full[i:i + 2])])
    d = 4 * q
    gs.append([(d, SPW * q, 512, 0), (d + 1, SPW * q + 128, 384, 512)])
    gs.append([(d + 2, SPW * q + 256, 256, 0), (d + 3, SPW * q + 384, 128, 256)])
    return gs


def _build(debug=False, reps=1):
    nc = bacc.Bacc()
    xT = nc.declare_dram_parameter("xT", [D, T], F32R, isOutput=False)
    Wa = nc.declare_dram_parameter("Wa", [D, 3 * HDIM], F32R, isOutput=False)
    Wp = nc.declare_dram_parameter("Wp", [HDIM, D], F32R, isOutput=False)
    bqk = nc.declare_dram_parameter("bqk", [128, 4], F32, isOutput=False)
    bv = nc.declare_dram_parameter("bv", [1, HDIM], F32R, isOutput=False)
    tri = nc.declare_dram_parameter("tri", [128, 128], F32R, isOutput=False)
    out = nc.declare_dram_parameter("out", [D, T], F32, isOutput=True)
    if debug:
        dbg_qkT = nc.declare_dram_parameter("dbg_qkT", [128, 4 * T], F32, isOutput=True)
        dbg_vsb = nc.declare_dram_parameter("dbg_vsb", [128, 16 * 512], F32, isOutput=True)
        dbg_yT = nc.declare_dram_parameter("dbg_yT", [128, 2 * T], F32, isOutput=True)
        dbg_rden = nc.declare_dram_parameter("dbg_rden", [HPC, T], F32, isOutput=True)

    with tile.TileContext(nc) as tc:
        with (
            tc.tile_pool(name="persist", bufs=1) as pp,
            tc.tile_pool(name="pq", bufs=1, space="PSUM") as pq,
            tc.tile_pool(name="psS", bufs=2, space="PSUM") as psS,
            tc.tile_pool(name="pvo", bufs=2, space="PSUM") as pvo,
        ):
            # [partition, chunk, token]: chunks 0-1 = Q^T, 2-3 = K^T
            qkT = pp.tile([128, 4, T], F32R)
            # V1 blocks per (t-chunk, head): even head [V|1@64|0], odd [0|1@32|0|V@64]
            vsb = pp.tile([128, 16, 4 * 128], F32R)
            yT = pp.tile([128, 2, T], F32R)
            wp = pp.tile([128, 2, D], F32R)
            trisb = pp.tile([128, 128], F32R)
            bqksb = pp.tile([128, 4], F32)
            bvsb = pp.tile([1, HDIM], F32R)
            onesr = pp.tile([1, T], F32R)

            # constants / weights that persist across timing-loop iterations
            nc.sync.dma_start(trisb[:], tri[:])
            nc.sync.dma_start(bqksb[:], bqk[:])
            nc.sync.dma_start(bvsb[:], bv[:])
            for c in range(2):
                nc.sync.dma_start(wp[:, c, :], Wp[c * 128 : (c + 1) * 128, :])
            nc.gpsimd.memset(onesr[:].bitcast(F32), 1.0)
            nc.gpsimd.memset(vsb[:].bitcast(F32), 0.0)
            # ones columns: even heads at 128j+64 (j=0,2), odd at 128j+32 (j=1,3)
            nc.gpsimd.memset(vsb[:, :, 64::256].bitcast(F32), 1.0)
            nc.gpsimd.memset(vsb[:, :, 160::256].bitcast(F32), 1.0)

            loop_ctx = tc.For_i(0, reps, 1) if reps > 1 else contextlib.nullcontext()
            with loop_ctx, contextlib.ExitStack() as stk:
                xp = stk.enter_context(tc.tile_pool(name="xp", bufs=16))
                wsp = stk.enter_context(tc.tile_pool(name="wsp", bufs=1))
                wk = stk.enter_context(tc.tile_pool(name="wk", bufs=3))
                nm = stk.enter_context(tc.tile_pool(name="nm", bufs=2))
                rbp = stk.enter_context(tc.tile_pool(name="rbp", bufs=2))
                obp = stk.enter_context(tc.tile_pool(name="obp", bufs=3))

                # ---- input DMAs (sp-major; xp pool slots throttle sp>=2) ----
                ws = []
                xs = {}
                for k in range(KCH):
                    wt = wsp.tile([128, 3 * HDIM], F32R, tag=f"w{k}")
                    nc.sync.dma_start(wt[:], Wa[k * 128 : (k + 1) * 128, :])
                    ws.append(wt)
                    xt = xp.tile([128, SPW], F32R, tag="x")
                    nc.sync.dma_start(xt[:], xT[k * 128 : (k + 1) * 128, 0:SPW])
                    xs[k, 0] = xt
                for sp in range(1, NSP):
                    for k in range(KCH):
                        xt = xp.tile([128, SPW], F32R, tag="x")
                        nc.sync.dma_start(
                            xt[:], xT[k * 128 : (k + 1) * 128, sp * SPW : (sp + 1) * SPW]
                        )
                        xs[k, sp] = xt

                for sp in range(NSP):
                    # ---- QK projection for span sp (k-ordered accumulation) ----
                    for mp in (0, 2):
                        accq = pq.tile([128, 1024], F32, tag="pq", name="accq")
                        for k in range(KCH):
                            for i, m in enumerate((mp, mp + 1)):
                                nc.tensor.matmul(
                                    accq[:, i * 512 : (i + 1) * 512],
                                    ws[k][:, m * 128 : (m + 1) * 128],
                                    xs[k, sp][:],
                                    start=(k == 0),
                                    stop=(k == KCH - 1),
                                )
                        for i, m in enumerate((mp, mp + 1)):
                            nc.any.tensor_scalar_add(
                                qkT[:, m, sp * SPW : (sp + 1) * SPW],
                                accq[:, i * 512 : (i + 1) * 512],
                                bqksb[:, m : m + 1],
                            )

                    # ---- V for t-chunks of span sp ----
                    for tl in range(4):
                        t = 4 * sp + tl
                        acc = pvo.tile([128, 512], F32, tag="o", name="accv")[:, 0:HDIM]
                        kord = [(t + i) % KCH for i in range(KCH)]
                        for i, k in enumerate(kord):
                            nc.tensor.matmul(
                                acc[:],
                                xs[k, sp][:, tl * 128 : (tl + 1) * 128],
                                ws[k][:, 2 * HDIM : 3 * HDIM],
                                start=(i == 0),
                                stop=False,
                            )
                        nc.tensor.matmul(
                            acc[:],
                            onesr[0:1, t * 128 : (t + 1) * 128],
                            bvsb[0:1, :],
                            start=False,
                            stop=True,
                        )
                        # scatter 4 heads into their V1 blocks (2 strided copies)
                        src_e = acc[:, 0:192].rearrange("p (h d) -> p h d", d=64)[:, ::2]
                        dst_e = vsb[:, t, 0:320].rearrange("p (h d) -> p h d", d=64)[:, ::4]
                        nc.any.tensor_copy(dst_e, src_e)
                        src_o = acc[:, 64:256].rearrange("p (h d) -> p h d", d=64)[:, ::2]
                        dst_o = vsb[:, t, 192:512].rearrange("p (h d) -> p h d", d=64)[:, ::4]
                        nc.any.tensor_copy(dst_o, src_o)

                    # ---- attention: query quarter sp, all heads ----
                    q0 = sp * SPW
                    ki_last = 4 * sp + 3
                    for j in range(HPC):
                        po = 64 * (j % 2)
                        qc, kc = j // 2, 2 + j // 2
                        dp = 64 if j % 2 == 0 else 32
                        oh = pvo.tile([128, 512], F32, tag="o", name="oh")
                        for grp in _groups(sp):
                            sps_t = psS.tile([128, 1024], F32, tag="s")
                            for (ki, s, w, off) in grp:
                                nc.tensor.matmul(
                                    sps_t[:, off : off + w],
                                    qkT[po : po + 64, kc, 128 * ki : 128 * ki + 128],
                                    qkT[po : po + 64, qc, s : s + w],
                                    start=True,
                                    stop=True,
                                )
                            ew = max(off + w for (_, _, w, off) in grp)
                            psb = wk.tile([128, 1024], F32R, tag="p")
                            nc.scalar.activation(
                                psb[:, :ew], sps_t[:, :ew], EXP, scale=0.125
                            )
                            for (ki, s, w, off) in grp:
                                if s == 128 * ki:  # diagonal block at span start
                                    nc.vector.tensor_tensor(
                                        psb[:, off : off + 128],
                                        psb[:, off : off + 128],
                                        trisb[:],
                                        MULT,
                                    )
                            for (ki, s, w, off) in grp:
                                nc.tensor.matmul(
                                    oh[:, s - q0 : s - q0 + w],
                                    vsb[:, ki, 128 * j : 128 * (j + 1)],
                                    psb[:, off : off + w],
                                    start=(ki == 0),
                                    stop=(ki == ki_last),
                                    skip_group_check=True,
                                )
                        # normalize: yT[d, q] = oh[d, q] / denom[q]
                        drec = nm.tile([1, 512], F32, tag="dc")
                        nc.vector.reciprocal(drec[0:1, :], oh[dp : dp + 1, :])
                        if debug:
                            nc.sync.dma_start(
                                dbg_rden[j : j + 1, q0 : q0 + SPW], drec[0:1, :]
                            )
                        rb = rbp.tile([128, 512], F32R, tag="rb")
                        nc.gpsimd.partition_broadcast(rb[:], drec[0:1, :].bitcast(F32R))
                        nc.vector.tensor_tensor(
                            yT[po : po + 64, qc, q0 : q0 + SPW],
                            oh[po : po + 64, :],
                            rb[po : po + 64, :],
                            MULT,
                        )

                    # ---- output projection for span sp ----
                    for m in range(8):
                        acc = pvo.tile([128, 512], F32, tag="o", name="accd")
                        for c in range(2):
                            nc.tensor.matmul(
                                acc[:],
                                wp[:, c, m * 128 : (m + 1) * 128],
                                yT[:, c, sp * SPW : (sp + 1) * SPW],
                                start=(c == 0),
                                stop=(c == 1),
                            )
                        obt = obp.tile([128, 512], F32, tag="obt")
                        if m % 2 == 0:
                            nc.vector.tensor_copy(obt[:], acc[:])
                        else:
                            nc.scalar.copy(obt[:], acc[:])
                        nc.sync.dma_start(
                            out[m * 128 : (m + 1) * 128, sp * SPW : (sp + 1) * SPW],
                            obt[:],
                        )

                if debug:
                    nc.sync.dma_start(dbg_qkT[:], qkT[:].bitcast(F32))
                    nc.sync.dma_start(dbg_vsb[:], vsb[:].bitcast(F32))
                    nc.sync.dma_start(dbg_yT[:], yT[:].bitcast(F32))

    nc.compile()
    return nc


def _get_nc():
    global _NC
    if _NC is None:
        _NC = _build()
    return _NC


def _make_in_maps(x, Wqkv, bqkv, Wproj):
    x = np.ascontiguousarray(np.asarray(x, np.float32))
    Wqkv = np.asarray(Wqkv, np.float32)
    bqkv = np.asarray(bqkv, np.float32)
    Wproj = np.asarray(Wproj, np.float32)
    tri = np.triu(np.ones((128, 128), np.float32))  # tri[k, q] = q >= k
    in_maps = []
    for c in range(NCORES):
        b = c // 4
        cs = (c % 4) * HDIM
        ce = cs + HDIM
        Wa = np.concatenate(
            [Wqkv[:, cs:ce], Wqkv[:, D + cs : D + ce], Wqkv[:, 2 * D + cs : 2 * D + ce]],
            axis=1,
        )
        bqk_c = np.concatenate([bqkv[cs:ce], bqkv[D + cs : D + ce]])
        in_maps.append(
            {
                "xT": np.ascontiguousarray(x[b].T),
                "Wa": np.ascontiguousarray(Wa),
                "Wp": np.ascontiguousarray(Wproj[cs:ce, :]),
                "bqk": np.ascontiguousarray(bqk_c.reshape(4, 128).T),
                "bv": np.ascontiguousarray(bqkv[2 * D + cs : 2 * D + ce].reshape(1, HDIM)),
                "tri": tri,
            }
        )
    return in_maps


def _run(in_maps, **kwargs):
    nc = _get_nc()
    return run_bass_kernel_spmd(nc, in_maps, core_ids=list(range(NCORES)), **kwargs)


def kernel(x, Wqkv, bqkv, Wproj, bproj):
    in_maps = _make_in_maps(x, Wqkv, bqkv, Wproj)
    res = _run(in_maps)
    bproj = np.asarray(bproj, np.float32)
    outp = np.zeros((B, T, D), np.float32)
    for c in range(NCORES):
        outp[c // 4] += res.results[c]["out"].T
    outp += bproj[None, None, :]
    return outp


# revision 4
# speedup vs baseline: 1.0336x; 1.0336x over previous
"""Causal self-attention (B=2, T=2048, D=1024, NH=16) on 8 Trainium2 NeuronCores.

Sharding: core c handles batch b = c // 4 and heads [4*(c%4), 4*(c%4)+4)
(data parallel over B, head/tensor parallel over NH). Wqkv is column-sliced,
Wproj row-sliced per core; each core emits its partial projection
out_c = Y_c @ Wproj_c (transposed, [D, T]); the host sums the four partials
per batch and adds bproj. No cross-core collectives are needed.

v2: the whole kernel is a 4-stage software pipeline over 512-token column
spans ("quarters"). Per span sp: x pieces stream in (DMA) -> QKV projection
for that span -> attention for query quarter sp (all 4 heads) -> output
projection for span sp -> output DMA. Stages overlap: while quarter sp's
softmax exps run on ACT, the tensor engine computes QK/V for span sp+1 and
the projection for span sp. This keeps PE (the overall bottleneck) and ACT
(the attention bottleneck) both fed, and hides the input-DMA ramp behind
k-ordered QK accumulation.

On-device layout (as v1): contraction dim on SBUF partitions everywhere.
Host passes x^T so QKV^T = W^T @ x^T lands in [dim, token] layout. Attention
computes S^T (keys on partitions); exp(S^T) feeds the PV matmul straight
from SBUF; the softmax denominator comes free via ones-columns in the V
blocks (V1 layout per 128-token chunk: even head [V|1@64|pad], odd head
[pad|1@32|pad|V@64]). Causality: skip all-masked key blocks, mask the
diagonal 128x128 block with a 0/1 triangular SBUF mask. All S-tile psum
slots are packed tightly (spans start mid-bank where needed) so exp never
processes garbage columns; all but the unavoidable 128-wide diagonal
matmuls keep free dim >= 256 (fp32r runs 1/4 rate below 256).

Matmuls run in float32r (TF32-like fast fp32 mode; ~1e-3 max rel err).
PSUM budget (8 banks): 2 banks QK pair accumulators (own pool so span
sp+1's QKV can run during span sp's attention), 4 banks S tiles
(2 x [128,1024], double-buffered so S matmuls overlap the previous
group's exp), 2 banks shared V accumulators / attention outputs /
projection accumulators. The causal masks for a diagonal group's two
key blocks are applied in one DVE multiply using a precomputed
[tri|1x384|tri|1x128|tri] mask strip.
"""

import contextlib

import numpy as np

import concourse.bass as bass
import concourse.mybir as mybir
import concourse.tile as tile
from concourse import bacc
from concourse.bass_utils import run_bass_kernel_spmd

B, T, D = 2, 2048, 1024
NH, HD = 16, 64
NCORES = 8
HPC = 4                 # heads per core
HDIM = HPC * HD         # 256 qkv dims per core
KCH = D // 128          # 8 contraction chunks
NSP = 4                 # 512-token column spans
SPW = T // NSP          # 512
F32 = mybir.dt.float32
F32R = mybir.dt.float32r
EXP = mybir.ActivationFunctionType.Exp
MULT = mybir.AluOpType.mult

_NC = None


def _groups(q):
    """S-block groups for query quarter q. Each group is a list of
    (ki, s, w, off): key block ki, query span [s, s+w), psum col offset off.
    Offsets are packed so exp covers no garbage columns; every span stays
    inside a 512-col psum bank."""
    gs = []
    full = list(range(0, 4 * q))
    for i in range(0, len(full), 2):
        gs.append([(ki, SPW * q, 512, 512 * idx)
                   for idx, ki in enumerate(full[i:i + 2])])
    d = 4 * q
    gs.append([(d, SPW * q, 512, 0), (d + 1, SPW * q + 128, 384, 512)])
    gs.append([(d + 2, SPW * q + 256, 256, 0), (d + 3, SPW * q + 384, 128, 256)])
    return gs


def _build(debug=False, reps=1):
    nc = bacc.Bacc()
    xT = nc.declare_dram_parameter("xT", [D, T], F32R, isOutput=False)
    Wa = nc.declare_dram_parameter("Wa", [D, 3 * HDIM], F32R, isOutput=False)
    Wp = nc.declare_dram_parameter("Wp", [HDIM, D], F32R, isOutput=False)
    bqk = nc.declare_dram_parameter("bqk", [128, 4], F32, isOutput=False)
    bv = nc.declare_dram_parameter("bv", [1, HDIM], F32R, isOutput=False)
    tri = nc.declare_dram_parameter("tri", [128, 896], F32R, isOutput=False)
    out = nc.declare_dram_parameter("out", [D, T], F32, isOutput=True)
    if debug:
        dbg_qkT = nc.declare_dram_parameter("dbg_qkT", [128, 4 * T], F32, isOutput=True)
        dbg_vsb = nc.declare_dram_parameter("dbg_vsb", [128, 16 * 512], F32, isOutput=True)
        dbg_yT = nc.declare_dram_parameter("dbg_yT", [128, 2 * T], F32, isOutput=True)
        dbg_rden = nc.declare_dram_parameter("dbg_rden", [HPC, T], F32, isOutput=True)

    with tile.TileContext(nc) as tc:
        with (
            tc.tile_pool(name="persist", bufs=1) as pp,
            tc.tile_pool(name="pq", bufs=1, space="PSUM") as pq,
            tc.tile_pool(name="psS", bufs=2, space="PSUM") as psS,
            tc.tile_pool(name="pvo", bufs=2, space="PSUM") as pvo,
        ):
            # [partition, chunk, token]: chunks 0-1 = Q^T, 2-3 = K^T
            qkT = pp.tile([128, 4, T], F32R)
            # V1 blocks per (t-chunk, head): even head [V|1@64|0], odd [0|1@32|0|V@64]
            vsb = pp.tile([128, 16, 4 * 128], F32R)
            yT = pp.tile([128, 2, T], F32R)
            wp = pp.tile([128, 2, D], F32R)
            trisb = pp.tile([128, 896], F32R)
            bqksb = pp.tile([128, 4], F32)
            bvsb = pp.tile([1, HDIM], F32R)
            onesr = pp.tile([1, T], F32R)

            # constants / weights that persist across timing-loop iterations
            nc.sync.dma_start(trisb[:], tri[:])
            nc.sync.dma_start(bqksb[:], bqk[:])
            nc.sync.dma_start(bvsb[:], bv[:])
            for c in range(2):
                nc.sync.dma_start(wp[:, c, :], Wp[c * 128 : (c + 1) * 128, :])
            nc.gpsimd.memset(onesr[:].bitcast(F32), 1.0)
            nc.gpsimd.memset(vsb[:].bitcast(F32), 0.0)
            # ones columns: even heads at 128j+64 (j=0,2), odd at 128j+32 (j=1,3)
            nc.gpsimd.memset(vsb[:, :, 64::256].bitcast(F32), 1.0)
            nc.gpsimd.memset(vsb[:, :, 160::256].bitcast(F32), 1.0)

            loop_ctx = tc.For_i(0, reps, 1) if reps > 1 else contextlib.nullcontext()
            with loop_ctx, contextlib.ExitStack() as stk:
                xp = stk.enter_context(tc.tile_pool(name="xp", bufs=16))
                wsp = stk.enter_context(tc.tile_pool(name="wsp", bufs=1))
                wk = stk.enter_context(tc.tile_pool(name="wk", bufs=3))
                nm = stk.enter_context(tc.tile_pool(name="nm", bufs=2))
                rbp = stk.enter_context(tc.tile_pool(name="rbp", bufs=2))
                obp = stk.enter_context(tc.tile_pool(name="obp", bufs=3))

                # ---- input DMAs (sp-major; xp pool slots throttle sp>=2) ----
                ws = []
                xs = {}
                for k in range(KCH):
                    wt = wsp.tile([128, 3 * HDIM], F32R, tag=f"w{k}")
                    nc.sync.dma_start(wt[:], Wa[k * 128 : (k + 1) * 128, :])
                    ws.append(wt)
                    xt = xp.tile([128, SPW], F32R, tag="x")
                    nc.sync.dma_start(xt[:], xT[k * 128 : (k + 1) * 128, 0:SPW])
                    xs[k, 0] = xt
                for sp in range(1, NSP):
                    for k in range(KCH):
                        xt = xp.tile([128, SPW], F32R, tag="x")
                        nc.sync.dma_start(
                            xt[:], xT[k * 128 : (k + 1) * 128, sp * SPW : (sp + 1) * SPW]
                        )
                        xs[k, sp] = xt

                for sp in range(NSP):
                    # ---- QK projection for span sp (k-ordered accumulation) ----
                    for mp in (0, 2):
                        accq = pq.tile([128, 1024], F32, tag="pq", name="accq")
                        for k in range(KCH):
                            for i, m in enumerate((mp, mp + 1)):
                                nc.tensor.matmul(
                                    accq[:, i * 512 : (i + 1) * 512],
                                    ws[k][:, m * 128 : (m + 1) * 128],
                                    xs[k, sp][:],
                                    start=(k == 0),
                                    stop=(k == KCH - 1),
                                )
                        for i, m in enumerate((mp, mp + 1)):
                            nc.any.tensor_scalar_add(
                                qkT[:, m, sp * SPW : (sp + 1) * SPW],
                                accq[:, i * 512 : (i + 1) * 512],
                                bqksb[:, m : m + 1],
                            )

                    # ---- V for t-chunks of span sp ----
                    for tl in range(4):
                        t = 4 * sp + tl
                        acc = pvo.tile([128, 512], F32, tag="o", name="accv")[:, 0:HDIM]
                        kord = [(t + i) % KCH for i in range(KCH)]
                        for i, k in enumerate(kord):
                            nc.tensor.matmul(
                                acc[:],
                                xs[k, sp][:, tl * 128 : (tl + 1) * 128],
                                ws[k][:, 2 * HDIM : 3 * HDIM],
                                start=(i == 0),
                                stop=False,
                            )
                        nc.tensor.matmul(
                            acc[:],
                            onesr[0:1, t * 128 : (t + 1) * 128],
                            bvsb[0:1, :],
                            start=False,
                            stop=True,
                        )
                        # scatter 4 heads into their V1 blocks (2 strided copies)
                        src_e = acc[:, 0:192].rearrange("p (h d) -> p h d", d=64)[:, ::2]
                        dst_e = vsb[:, t, 0:320].rearrange("p (h d) -> p h d", d=64)[:, ::4]
                        nc.any.tensor_copy(dst_e, src_e)
                        src_o = acc[:, 64:256].rearrange("p (h d) -> p h d", d=64)[:, ::2]
                        dst_o = vsb[:, t, 192:512].rearrange("p (h d) -> p h d", d=64)[:, ::4]
                        nc.any.tensor_copy(dst_o, src_o)

                    # ---- attention: query quarter sp, all heads ----
                    q0 = sp * SPW
                    ki_last = 4 * sp + 3
                    for j in range(HPC):
                        po = 64 * (j % 2)
                        qc, kc = j // 2, 2 + j // 2
                        dp = 64 if j % 2 == 0 else 32
                        oh = pvo.tile([128, 512], F32, tag="o", name="oh")
                        for grp in _groups(sp):
                            sps_t = psS.tile([128, 1024], F32, tag="s")
                            for (ki, s, w, off) in grp:
                                nc.tensor.matmul(
                                    sps_t[:, off : off + w],
                                    qkT[po : po + 64, kc, 128 * ki : 128 * ki + 128],
                                    qkT[po : po + 64, qc, s : s + w],
                                    start=True,
                                    stop=True,
                                )
                            ew = max(off + w for (_, _, w, off) in grp)
                            psb = wk.tile([128, 1024], F32R, tag="p")
                            nc.scalar.activation(
                                psb[:, :ew], sps_t[:, :ew], EXP, scale=0.125
                            )
                            # diagonal groups: one mask multiply over the
                            # whole group (mask slices: [tri|1x384|tri|1x128|tri])
                            if grp[0][1] == 128 * grp[0][0]:
                                if grp[0][2] == 512:   # group (512, 384)
                                    nc.vector.tensor_tensor(
                                        psb[:, 0:640], psb[:, 0:640],
                                        trisb[:, 0:640], MULT,
                                    )
                                else:                  # group (256, 128)
                                    nc.vector.tensor_tensor(
                                        psb[:, 0:384], psb[:, 0:384],
                                        trisb[:, 512:896], MULT,
                                    )
                            for (ki, s, w, off) in grp:
                                nc.tensor.matmul(
                                    oh[:, s - q0 : s - q0 + w],
                                    vsb[:, ki, 128 * j : 128 * (j + 1)],
                                    psb[:, off : off + w],
                                    start=(ki == 0),
                                    stop=(ki == ki_last),
                                    skip_group_check=True,
                                )
                        # normalize: yT[d, q] = oh[d, q] / denom[q]
                        drec = nm.tile([1, 512], F32, tag="dc")
                        nc.vector.reciprocal(drec[0:1, :], oh[dp : dp + 1, :])
                        if debug:
                            nc.sync.dma_start(
                                dbg_rden[j : j + 1, q0 : q0 + SPW], drec[0:1, :]
                            )
                        rb = rbp.tile([128, 512], F32R, tag="rb")
                        nc.gpsimd.partition_broadcast(rb[:], drec[0:1, :].bitcast(F32R))
                        nc.vector.tensor_tensor(
                            yT[po : po + 64, qc, q0 : q0 + SPW],
                            oh[po : po + 64, :],
                            rb[po : po + 64, :],
                            MULT,
                        )

                    # ---- output projection for span sp ----
                    for m in range(8):
                        acc = pvo.tile([128, 512], F32, tag="o", name="accd")
                        for c in range(2):
                            nc.tensor.matmul(
                                acc[:],
                                wp[:, c, m * 128 : (m + 1) * 128],
                                yT[:, c, sp * SPW : (sp + 1) * SPW],
                                start=(c == 0),
                                stop=(c == 1),
                            )
                        obt = obp.tile([128, 512], F32, tag="obt")
                        if m % 2 == 0:
                            nc.vector.tensor_copy(obt[:], acc[:])
                        else:
                            nc.scalar.copy(obt[:], acc[:])
                        nc.sync.dma_start(
                            out[m * 128 : (m + 1) * 128, sp * SPW : (sp + 1) * SPW],
                            obt[:],
                        )

                if debug:
                    nc.sync.dma_start(dbg_qkT[:], qkT[:].bitcast(F32))
                    nc.sync.dma_start(dbg_vsb[:], vsb[:].bitcast(F32))
                    nc.sync.dma_start(dbg_yT[:], yT[:].bitcast(F32))

    nc.compile()
    return nc


def _get_nc():
    global _NC
    if _NC is None:
        _NC = _build()
    return _NC


def _make_in_maps(x, Wqkv, bqkv, Wproj):
    x = np.ascontiguousarray(np.asarray(x, np.float32))
    Wqkv = np.asarray(Wqkv, np.float32)
    bqkv = np.asarray(bqkv, np.float32)
    Wproj = np.asarray(Wproj, np.float32)
    t128 = np.triu(np.ones((128, 128), np.float32))  # tri[k, q] = q >= k
    ones = np.ones((128, 384), np.float32)
    tri = np.concatenate([t128, ones, t128, ones[:, :128], t128], axis=1)
    in_maps = []
    for c in range(NCORES):
        b = c // 4
        cs = (c % 4) * HDIM
        ce = cs + HDIM
        Wa = np.concatenate(
            [Wqkv[:, cs:ce], Wqkv[:, D + cs : D + ce], Wqkv[:, 2 * D + cs : 2 * D + ce]],
            axis=1,
        )
        bqk_c = np.concatenate([bqkv[cs:ce], bqkv[D + cs : D + ce]])
        in_maps.append(
            {
                "xT": np.ascontiguousarray(x[b].T),
                "Wa": np.ascontiguousarray(Wa),
                "Wp": np.ascontiguousarray(Wproj[cs:ce, :]),
                "bqk": np.ascontiguousarray(bqk_c.reshape(4, 128).T),
                "bv": np.ascontiguousarray(bqkv[2 * D + cs : 2 * D + ce].reshape(1, HDIM)),
                "tri": tri,
            }
        )
    return in_maps


def _run(in_maps, **kwargs):
    nc = _get_nc()
    return run_bass_kernel_spmd(nc, in_maps, core_ids=list(range(NCORES)), **kwargs)


def kernel(x, Wqkv, bqkv, Wproj, bproj):
    in_maps = _make_in_maps(x, Wqkv, bqkv, Wproj)
    res = _run(in_maps)
    bproj = np.asarray(bproj, np.float32)
    outp = np.zeros((B, T, D), np.float32)
    for c in range(NCORES):
        outp[c // 4] += res.results[c]["out"].T
    outp += bproj[None, None, :]
    return outp
